# revision 3
# baseline (speedup 1.0000x reference)
"""Trainium2 Bass kernel for DenseNet + PWLNN (5-NN over 250 centers) — V2.

Contract: kernel(**inputs) takes FULL inputs (as in reference.setup_inputs())
and returns the FULL output [524288, 2] float32.

V2 strategy (vs baseline): pure data parallel over 8 cores, 65536 samples
each, 128 blocks of 512 samples, processed in 32 groups of 4 blocks.

Per group:
  - one x DMA [32, 2048] into a persistent H tile [58, 2048]
    (row 0 = ones, rows 1-32 = x, rows 33-57 = tanh features),
  - 5 dense layers: per layer ONE matmul per block (fp32r, 1 cyc/row) into a
    shared PSUM bank at partition offsets {0,32,64,96}, ONE tanh activation
    covering all 4 blocks ([101, 512] -> staging tile), ONE SBUF->SBUF DMA
    consolidating the 4 blocks' 5 tanh rows back into H (20 descriptors),
  - per block: 4 score matmuls (fp32r, caug [58, 256]: 250 center scores +
    4 duplicated enc columns), top-8 via DVE Max, midpoint threshold,
    Sign (+/-1, bf16) on ScalarE, bf16 PE transposes, one bf16 PSUM->SBUF
    copy, 2 bf16 G-matmuls (with a folded q row via a preset ones column),
    3 batched DVE ops for the final affine combine,
  - output staged [128, 32, 2] and DMA'd p-major every 8 blocks (host
    reorders at the end).

Emission is software-pipelined: block phases of group g-1 are interleaved
into the layer slots of group g so PE/Act/DVE always have ready work.
"""

import sys
import numpy as np

sys.path.insert(0, "/opt/trn_rl_repo")

N_SMPS = 524288
N_CORES = 8
N_PER = N_SMPS // N_CORES  # 65536
D_IN = 32
GROWTH = 5
N_LAYERS = 5
D_H = D_IN + N_LAYERS * GROWTH  # 57
D_ENC = 2
N_FCNS = 250
BLK = 512
NBLK = N_PER // BLK  # 128
GRP = 2  # blocks per group
NGRP = NBLK // GRP  # 32
CH = 128
NCH = BLK // CH  # 4
NSC = 256  # padded score columns: 250 scores + 4 enc-dup + 2 pad
HB_RING = 10
S_RING = 4

_cache = {}


def _build_program(nblk=NBLK, reps=1):
    import concourse.bass as bass
    import concourse.tile as tile
    import concourse.tile_sem_assignment as _tsa
    from concourse import mybir

    _tsa.NUM_SWDGE_GLOBAL_SEMS = 1

    f32 = mybir.dt.float32
    f32r = mybir.dt.float32r
    bf16 = mybir.dt.bfloat16
    fp16 = mybir.dt.float16
    AF = mybir.ActivationFunctionType

    ngrp = nblk // GRP

    nc = bass.Bass()

    x_fm = nc.dram_tensor("x_fm", [D_IN, N_PER], f32, kind="ExternalInput")
    wt_d = [
        nc.dram_tensor(f"wt{i}", [33 + 5 * i, 32], f32, kind="ExternalInput")
        for i in range(N_LAYERS)
    ]
    caug_d = nc.dram_tensor("caug2", [D_H + 1, NSC], f32, kind="ExternalInput")
    g1_d = nc.dram_tensor("g1b", [128, 6], fp16, kind="ExternalInput")
    g2_d = nc.dram_tensor("g2b", [123, 6], fp16, kind="ExternalInput")
    id_d = nc.dram_tensor("identb", [128, 128], fp16, kind="ExternalInput")
    ones_d = nc.dram_tensor("ones1024", [1, GRP * BLK], f32, kind="ExternalInput")
    out_d = nc.dram_tensor("out", [CH, NBLK * NCH, D_ENC], f32, kind="ExternalOutput")

    with tile.TileContext(nc) as tc:
        with (
            tc.tile_pool(name="const", bufs=1) as constp,
            tc.tile_pool(name="ht", bufs=2) as htp,
            tc.tile_pool(name="st", bufs=3) as stp,
            tc.tile_pool(name="small", bufs=4) as smallp,
            tc.tile_pool(name="outp", bufs=2) as outp,
            tc.tile_pool(name="pz", bufs=1, space=bass.MemorySpace.PSUM) as pzp,
            tc.tile_pool(name="psm", bufs=3, space=bass.MemorySpace.PSUM) as psmp,
            tc.tile_pool(name="pst", bufs=2, space=bass.MemorySpace.PSUM) as pstp,
            tc.tile_pool(name="pt4", bufs=1, space=bass.MemorySpace.PSUM) as pt4p,
        ):
            # ---- constants ----
            wt = []
            for i in range(N_LAYERS):
                t = constp.tile([33 + 5 * i, 32], f32, tag=f"wt{i}")
                nc.sync.dma_start(t[:], wt_d[i][:])
                wt.append(t)
            caug = constp.tile([D_H + 1, NSC], f32, tag="caug")
            nc.sync.dma_start(caug[:], caug_d[:])
            g1b = constp.tile([128, 6], fp16, tag="g1b")
            nc.sync.dma_start(g1b[:], g1_d[:])
            g2b = constp.tile([123, 6], fp16, tag="g2b")
            nc.sync.dma_start(g2b[:], g2_d[:])
            identb = constp.tile([128, 128], fp16, tag="identb")
            nc.sync.dma_start(identb[:], id_d[:])

            # Persistent rings: H tiles (ones row preset), S tiles (mask pad
            # columns preset: col 250 = 1 feeds the folded q row, 251+ = 0).
            hb_ring = []
            for r in range(HB_RING):
                t = constp.tile([D_H + 1, GRP * BLK], f32, tag=f"hb{r}")
                nc.sync.dma_start(t[0:1, :], ones_d[:])
                hb_ring.append(t)
            s_ring = []
            for r in range(S_RING):
                t = constp.tile([CH, 2, NSC], fp16, tag=f"sr{r}")
                nc.gpsimd.memset(t[:, :, N_FCNS : N_FCNS + 1], 1.0)
                nc.gpsimd.memset(t[:, :, N_FCNS + 1 : NSC], 0.0)
                s_ring.append(t)

            state = {"outstage": None}

            def phase_a(g, j, h):
                """Scores / top-k / threshold / mask for chunks (2h, 2h+1)
                of block b = GRP*g + j."""
                b = GRP * g + j
                hb = hb_ring[g % HB_RING]
                psm = psmp.tile([CH, 2, NSC], f32, tag="psm")
                for c in range(2):
                    ck = 2 * h + c
                    nc.tensor.matmul(
                        psm[:, c, :],
                        hb[:, j * BLK + ck * CH : j * BLK + (ck + 1) * CH],
                        caug[:],
                        start=True,
                        stop=True,
                    )
                top32 = smallp.tile([CH, 2, 8], f32, tag="top32")
                for c in range(2):
                    nc.vector.max(top32[:, c, :], psm[:, c, 0:N_FCNS])
                tsum = smallp.tile([CH, 2, 1], f32, tag="tsum")
                nc.gpsimd.tensor_add(tsum[:], top32[:, :, 4:5], top32[:, :, 5:6])
                bias4 = smallp.tile([CH, 2, 1], f32, tag="bias4")
                nc.gpsimd.tensor_scalar_mul(bias4[:], tsum[:], -0.5)
                ecp = smallp.tile([CH, 2, 4], f32, tag="ecp")
                nc.vector.tensor_copy(ecp[:], psm[:, :, 250:254])

                S = s_ring[(2 * b + h) % S_RING]
                for c in range(2):
                    nc.scalar.activation(
                        S[:, c, 0:N_FCNS],
                        psm[:, c, 0:N_FCNS],
                        AF.Sign,
                        bias=bias4[:, c, :],
                    )
                return (S, ecp)

            def phase_b(g, j, h, S, ecp):
                """Transposes / G matmuls / combine for chunks (2h, 2h+1)."""
                b = GRP * g + j
                pt4 = pt4p.tile([CH, 2, 6], f32, tag="pt4")
                for c in range(2):
                    pst = pstp.tile([128, 256], fp16, tag="pst")
                    nc.tensor.transpose(pst[:, 0:128], S[:, c, 0:128], identb[:])
                    nc.tensor.transpose(pst[:, 128:256], S[:, c, 128:256], identb[:])
                    st = stp.tile([128, 256], fp16, tag="st")
                    if c == 0:
                        nc.scalar.copy(st[:], pst[:])
                    else:
                        nc.vector.tensor_copy(st[:], pst[:])
                    nc.tensor.matmul(
                        pt4[:, c, :], st[:, 0:128], g1b[:], start=True, stop=False
                    )
                    nc.tensor.matmul(
                        pt4[:, c, :],
                        st[0:123, 128:256],
                        g2b[:],
                        start=False,
                        stop=True,
                    )
                v = smallp.tile([CH, 2, 4], f32, tag="v")
                nc.vector.tensor_mul(v[:], pt4[:, :, 0:4], ecp[:])
                w = smallp.tile([CH, 2, 2], f32, tag="w")
                nc.vector.tensor_add(w[:], v[:, :, 0:2], v[:, :, 2:4])
                m = b % 8
                if m == 0 and h == 0:
                    outstage = outp.tile([CH, 8 * NCH, D_ENC], f32, tag="os")
                    state["outstage"] = outstage
                outstage = state["outstage"]
                nc.vector.scalar_tensor_tensor(
                    outstage[:, 4 * m + 2 * h : 4 * m + 2 * h + 2, :],
                    pt4[:, :, 4:6],
                    2.0 ** -10,
                    w[:],
                    mybir.AluOpType.mult,
                    mybir.AluOpType.add,
                )
                if m == 7 and h == 1:
                    g8 = b // 8
                    nc.sync.dma_start(
                        out_d[:, 32 * g8 : 32 * (g8 + 1), :], outstage[:]
                    )

            import contextlib
            from collections import deque

            QUAD = 4  # groups per pipeline wave
            nquad = ngrp // QUAD

            loop_cm = tc.For_i(0, reps, 1) if reps > 1 else contextlib.nullcontext()
            with loop_cm:
                pending = deque()
                pending_b = deque()
                for t_ in range(nquad + 1):
                    quad = list(range(t_ * QUAD, min((t_ + 1) * QUAD, ngrp)))
                    if t_ < nquad:
                        for g in quad:
                            hb = hb_ring[g % HB_RING]
                            nc.sync.dma_start(
                                hb[1 : 1 + D_IN, :],
                                x_fm[:, g * GRP * BLK : (g + 1) * GRP * BLK],
                            )
                        for i in range(N_LAYERS):
                            fi = 33 + 5 * i
                            pz4 = pzp.tile([128, GRP, BLK], f32, tag="pz")
                            for q, g in enumerate(quad):
                                hb = hb_ring[g % HB_RING]
                                for j in range(GRP):
                                    nc.tensor.matmul(
                                        pz4[32 * q : 32 * q + 32, j, :],
                                        wt[i][:],
                                        hb[0:fi, j * BLK : (j + 1) * BLK],
                                        start=True,
                                        stop=True,
                                        tile_position=(0, 32 * q),
                                    )
                            ht4 = htp.tile([101, GRP * BLK], f32, tag="ht")
                            nc.scalar.activation(
                                ht4[:],
                                pz4[0:101, :, :].rearrange("p a b -> p (a b)"),
                                AF.Tanh,
                            )
                            for q, g in enumerate(quad):
                                hb = hb_ring[g % HB_RING]
                                nc.sync.dma_start(
                                    hb[fi : fi + 5, :], ht4[32 * q : 32 * q + 5, :]
                                )
                            # interleave prior quad's block phases into this
                            # slot; stage B runs one phase behind stage A
                            for _ in range(4):
                                if pending:
                                    key = pending.popleft()
                                    stt = phase_a(*key)
                                    pending_b.append((key, stt))
                                if len(pending_b) > 1:
                                    key2, st2 = pending_b.popleft()
                                    phase_b(*key2, *st2)
                    # flush remaining phases of the prior quad
                    while pending:
                        key = pending.popleft()
                        stt = phase_a(*key)
                        pending_b.append((key, stt))
                        if len(pending_b) > 1:
                            key2, st2 = pending_b.popleft()
                            phase_b(*key2, *st2)
                    while pending_b:
                        key2, st2 = pending_b.popleft()
                        phase_b(*key2, *st2)
                    if t_ < nquad:
                        for g in quad:
                            for j in range(GRP):
                                for h in range(2):
                                    pending.append((g, j, h))

    _split_multi_waits(nc, mybir)
    return nc


def _split_multi_waits(nc, mybir):
    """walrus codegen allows only one sync-wait per instruction; hoist extra
    waits into standalone EventSemaphore instructions on the same engine."""
    k = 0
    for f in nc.m.functions:
        for blk in f.blocks:
            newl = []
            changed = False
            for ins in blk.instructions:
                si = ins.sync_info
                if si is not None and len(si.on_wait) > 1:
                    waits = list(si.on_wait)
                    for w in waits[:-1]:
                        ev = mybir.InstEventSemaphore(
                            name=f"WSPLIT-{k}", ins=[], outs=[]
                        )
                        k += 1
                        ev.engine = ins.engine
                        ev.sync_info = mybir.SyncInfo(on_wait=[w], on_update=[])
                        newl.append(ev)
                    ins.sync_info = mybir.SyncInfo(
                        on_wait=[waits[-1]], on_update=list(si.on_update)
                    )
                    changed = True
                newl.append(ins)
            if changed:
                blk.instructions = newl


def _host_constants(W_list, Wout, bout, ctrs, wts, offsets):
    """Folded constant tables (float64 accumulation, f32/bf16 output)."""
    import ml_dtypes

    ctrs64 = ctrs.astype(np.float64)
    Wout64 = Wout.astype(np.float64)
    bout64 = bout.astype(np.float64)
    wts64 = wts.astype(np.float64)
    off64 = offsets.astype(np.float64)

    # caug in baseline feature order: rows 0..56 = features, 57 = ones row.
    caug_old = np.zeros((D_H + 1, NSC), dtype=np.float64)
    caug_old[0:D_H, 0:N_FCNS] = 2.0 * (Wout64.T @ ctrs64.T)
    rrow = 2.0 * (ctrs64 @ bout64) - np.sum(ctrs64 * ctrs64, axis=1)
    caug_old[D_H, 0:N_FCNS] = rrow
    # enc-dup columns [e0, e0, e1, e1]
    sc = 2.0 ** -10
    caug_old[0:D_H, 250] = sc * Wout64.T[:, 0]
    caug_old[0:D_H, 251] = sc * Wout64.T[:, 0]
    caug_old[0:D_H, 252] = sc * Wout64.T[:, 1]
    caug_old[0:D_H, 253] = sc * Wout64.T[:, 1]
    caug_old[D_H, 250] = sc * bout64[0]
    caug_old[D_H, 251] = sc * bout64[0]
    caug_old[D_H, 252] = sc * bout64[1]
    caug_old[D_H, 253] = sc * bout64[1]

    # Reorder rows to the V2 H layout: [ones, x(32), tanh(25)].
    caug2 = np.zeros((D_H + 1, NSC), dtype=np.float32)
    caug2[0] = caug_old[D_H].astype(np.float32)
    caug2[1:33] = caug_old[0:32].astype(np.float32)
    caug2[33:58] = caug_old[32:57].astype(np.float32)

    # G[c] = [w00, w01, w10, w11, b'0, b'1]; b'_o = off[c,o] - sum_i w[c,i,o]*ctr[c,i]
    G = np.zeros((N_FCNS, 6), dtype=np.float64)
    G[:, 0] = wts64[:, 0, 0]
    G[:, 1] = wts64[:, 0, 1]
    G[:, 2] = wts64[:, 1, 0]
    G[:, 3] = wts64[:, 1, 1]
    bprime = off64 - np.einsum("cio,ci->co", wts64, ctrs64)
    G[:, 4] = bprime[:, 0]
    G[:, 5] = bprime[:, 1]

    g1 = (0.5 * 1024.0 * G[0:128]).astype(np.float16)
    g2 = np.zeros((123, 6), dtype=np.float16)
    g2[0:122] = (0.5 * 1024.0 * G[128:N_FCNS]).astype(np.float16)
    g2[122] = (0.5 * 1024.0 * G.sum(axis=0)).astype(np.float16)

    consts = {
        "caug2": caug2,
        "g1b": np.ascontiguousarray(g1),
        "g2b": np.ascontiguousarray(g2),
        "identb": np.eye(128, dtype=np.float16),
        "ones1024": np.ones((1, GRP * BLK), dtype=np.float32),
    }
    # Layer weight tiles [33+5i, 32]: row 0 = ones-row weight (0), rows 1-32 =
    # x weights, rows 33.. = tanh weights; cols 0-4 real, rest 0.
    for i, W in enumerate(W_list):
        fan_in = D_IN + i * GROWTH
        t = np.zeros((33 + 5 * i, 32), dtype=np.float32)
        t[1:33, 0:GROWTH] = W[:, 0:D_IN].astype(np.float32).T
        if i > 0:
            t[33 : 33 + 5 * i, 0:GROWTH] = W[:, D_IN:fan_in].astype(np.float32).T
        consts[f"wt{i}"] = t
    return consts


def _in_maps(inputs):
    x = np.asarray(inputs["x"], dtype=np.float32)
    W_list = [np.asarray(inputs[f"W{i}"], dtype=np.float32) for i in range(N_LAYERS)]
    consts = _host_constants(
        W_list,
        np.asarray(inputs["Wout"], dtype=np.float32),
        np.asarray(inputs["bout"], dtype=np.float32),
        np.asarray(inputs["ctrs"], dtype=np.float32),
        np.asarray(inputs["wts"], dtype=np.float32),
        np.asarray(inputs["offsets"], dtype=np.float32),
    )

    x_fm = np.ascontiguousarray(x.T)  # [32, N_SMPS]
    in_maps = []
    for core in range(N_CORES):
        m = dict(consts)
        m["x_fm"] = np.ascontiguousarray(x_fm[:, core * N_PER : (core + 1) * N_PER])
        in_maps.append(m)
    return in_maps


def _unstage(arr):
    """[128, 512, 2] p-major stage layout -> [65536, 2] sample-major."""
    return np.ascontiguousarray(
        arr.reshape(CH, NBLK, NCH, D_ENC).transpose(1, 2, 0, 3).reshape(N_PER, D_ENC)
    )


def _run(inputs, trace=False, nblk=NBLK):
    from concourse.bass_utils import run_bass_kernel_spmd

    key = ("nc", nblk)
    if key not in _cache:
        _cache[key] = _build_program(nblk)
    nc = _cache[key]

    in_maps = _in_maps(inputs)

    res = run_bass_kernel_spmd(nc, in_maps, list(range(N_CORES)), trace=trace)
    outs = [_unstage(res.results[c]["out"]) for c in range(N_CORES)]
    full = np.concatenate(outs, axis=0).astype(np.float32)
    return full, res


def kernel(**inputs):
    full, _ = _run(inputs, trace=False)
    return full


if __name__ == "__main__":
    rng = np.random.default_rng(0)
    demo = {"x": rng.standard_normal((N_SMPS, D_IN), dtype=np.float32)}
    for i in range(N_LAYERS):
        fan_in = D_IN + i * GROWTH
        demo[f"W{i}"] = rng.standard_normal((GROWTH, fan_in), dtype=np.float32) * 0.1
    demo["Wout"] = rng.standard_normal((D_ENC, D_H), dtype=np.float32) * 0.1
    demo["bout"] = rng.standard_normal(D_ENC, dtype=np.float32) * 0.1
    demo["ctrs"] = rng.standard_normal((N_FCNS, D_ENC), dtype=np.float32)
    demo["wts"] = 1e-5 * rng.standard_normal((N_FCNS, D_ENC, D_ENC), dtype=np.float32)
    demo["offsets"] = 1e-5 * rng.standard_normal((N_FCNS, D_ENC), dtype=np.float32)
    out = kernel(**demo)
    print(out.shape, out.dtype)


# revision 5
# speedup vs baseline: 1.5570x; 1.5570x over previous
"""Trainium2 Bass kernel for DenseNet + PWLNN (5-NN over 250 centers).

Contract: kernel(**inputs) takes FULL inputs (as in reference.setup_inputs())
and returns the FULL output [524288, 2] float32.

Strategy: pure data parallel over 8 NeuronCores (65536 samples each),
128 blocks of 512 samples per core, processed as a software-pipelined wave
of groups (GRP blocks per group, QUAD groups per wave):

  - H tile [58, GRP*512] per group: row 0 = ones, rows 1-32 = x (one DMA),
    rows 33-57 = tanh features, consolidated per layer by SBUF->SBUF DMA.
  - Dense layers in exact fp32: one matmul per (group, block, layer) into a
    shared PSUM bank at partition offsets {0,32,64,96} (fp32 allows nonzero
    tile positions; fp32r does not), ONE batched tanh activation per layer
    covering 4 groups ([101, GRP*512]).
  - Per half-block (2 chunks of 128 samples), stage A: 2 fp32 score matmuls
    vs caug [58, 256] (250 center scores + 4 enc-dup columns scaled 2^-10),
    top-8 via DVE Max, midpoint threshold on GPSIMD, Sign (+/-1, fp16) on
    ScalarE; stage B (emitted one phase later): fp16 PE transposes, one
    PSUM->SBUF copy, 2 fp16 G-matmuls (tables scaled 2^10 with a folded
    q row via a preset ones column), batched DVE combine ops.
  - Output staged [128, 32, 2] p-major, one DMA per 8 blocks; the host
    reorders to sample-major at the end.

Stage-A/stage-B phases of the previous wave are interleaved into the layer
slots of the current wave so PE/Act/DVE always have ready work despite the
serial matmul->tanh->DMA layer chain.

Precision notes (measured on HW): fp32r matmuls have only ~10 mantissa bits,
which flips near-tie 5th/6th neighbors for ~45% of samples (rel err 4e-2);
the top-5 selection tolerates only ~3e-5 absolute enc error, so the whole
DenseNet/score chain runs in true fp32. G tables in fp16 scaled by 2^10
(values ~1e-5 are fp16 denormals unscaled); masks are exact +/-1 in fp16.
"""

import sys
import numpy as np

sys.path.insert(0, "/opt/trn_rl_repo")

N_SMPS = 524288
N_CORES = 8
N_PER = N_SMPS // N_CORES  # 65536
D_IN = 32
GROWTH = 5
N_LAYERS = 5
D_H = D_IN + N_LAYERS * GROWTH  # 57
D_ENC = 2
N_FCNS = 250
BLK = 512
NBLK = N_PER // BLK  # 128
GRP = 2  # blocks per group
NGRP = NBLK // GRP  # 32
CH = 128
NCH = BLK // CH  # 4
NSC = 256  # padded score columns: 250 scores + 4 enc-dup + 2 pad
HB_RING = 18
S_RING = 6

_cache = {}


def _build_program(nblk=NBLK, reps=1):
    import concourse.bass as bass
    import concourse.tile as tile
    import concourse.tile_sem_assignment as _tsa
    from concourse import mybir

    _tsa.NUM_SWDGE_GLOBAL_SEMS = 1

    f32 = mybir.dt.float32
    f32r = mybir.dt.float32r
    bf16 = mybir.dt.bfloat16
    fp16 = mybir.dt.float16
    AF = mybir.ActivationFunctionType

    ngrp = nblk // GRP

    nc = bass.Bass()

    x_fm = nc.dram_tensor("x_fm", [2 * D_IN, N_PER], fp16, kind="ExternalInput")
    wt_d = [
        nc.dram_tensor(f"wt{i}", [66 + 10 * i, 32], fp16, kind="ExternalInput")
        for i in range(N_LAYERS)
    ]
    wl_d = [
        nc.dram_tensor(f"wl{i}", [66 + 10 * i, 32], fp16, kind="ExternalInput")
        for i in range(N_LAYERS)
    ]
    caug_d = nc.dram_tensor("cdup1", [2 * (D_H + 1), NSC], fp16, kind="ExternalInput")
    caug2_d = nc.dram_tensor("cdup2", [2 * (D_H + 1), NSC], fp16, kind="ExternalInput")
    g1_d = nc.dram_tensor("g1b", [128, 6], fp16, kind="ExternalInput")
    g2_d = nc.dram_tensor("g2b", [123, 6], fp16, kind="ExternalInput")
    id_d = nc.dram_tensor("identb", [128, 128], fp16, kind="ExternalInput")
    ones_d = nc.dram_tensor("ones1024", [2, GRP * BLK], fp16, kind="ExternalInput")
    out_d = nc.dram_tensor("out", [CH, NBLK * NCH, D_ENC], f32, kind="ExternalOutput")

    with tile.TileContext(nc) as tc:
        with (
            tc.tile_pool(name="const", bufs=1) as constp,
            tc.tile_pool(name="ht", bufs=3) as htp,
            tc.tile_pool(name="htf", bufs=2) as htfp,
            tc.tile_pool(name="htl", bufs=2) as htlp,
            tc.tile_pool(name="st", bufs=4) as stp,
            tc.tile_pool(name="small", bufs=6) as smallp,
            tc.tile_pool(name="outp", bufs=2) as outp,
            tc.tile_pool(name="pz", bufs=2, space=bass.MemorySpace.PSUM) as pzp,
            tc.tile_pool(name="psm", bufs=2, space=bass.MemorySpace.PSUM) as psmp,
            tc.tile_pool(name="pst", bufs=1, space=bass.MemorySpace.PSUM) as pstp,
            tc.tile_pool(name="pt4", bufs=1, space=bass.MemorySpace.PSUM) as pt4p,
        ):
            # ---- constants ----
            wt = []
            wl = []
            for i in range(N_LAYERS):
                t = constp.tile([66 + 10 * i, 32], fp16, tag=f"wt{i}")
                nc.sync.dma_start(t[:], wt_d[i][:])
                wt.append(t)
                t2 = constp.tile([66 + 10 * i, 32], fp16, tag=f"wl{i}")
                nc.sync.dma_start(t2[:], wl_d[i][:])
                wl.append(t2)
            caug = constp.tile([2 * (D_H + 1), NSC], fp16, tag="caug")
            nc.sync.dma_start(caug[:], caug_d[:])
            caug2 = constp.tile([2 * (D_H + 1), NSC], fp16, tag="caug2")
            nc.sync.dma_start(caug2[:], caug2_d[:])
            g1b = constp.tile([128, 6], fp16, tag="g1b")
            nc.sync.dma_start(g1b[:], g1_d[:])
            g2b = constp.tile([123, 6], fp16, tag="g2b")
            nc.sync.dma_start(g2b[:], g2_d[:])
            identb = constp.tile([128, 128], fp16, tag="identb")
            nc.sync.dma_start(identb[:], id_d[:])

            # Persistent rings: H tiles (ones row preset), S tiles (mask pad
            # columns preset: col 250 = 1 feeds the folded q row, 251+ = 0).
            hb_ring = []
            for r in range(HB_RING):
                t = constp.tile([2 * (D_H + 1), GRP * BLK], fp16, tag=f"hb{r}")
                nc.sync.dma_start(t[0:2, :], ones_d[:])
                hb_ring.append(t)
            s_ring = []
            for r in range(S_RING):
                t = constp.tile([CH, 2, NSC], fp16, tag=f"sr{r}")
                nc.gpsimd.memset(t[:, :, N_FCNS : N_FCNS + 1], 1.0)
                nc.gpsimd.memset(t[:, :, N_FCNS + 1 : NSC], 0.0)
                s_ring.append(t)

            state = {"outstage": None}

            def phase_a(g, j, h):
                """Scores / top-k / threshold / mask for chunks (2h, 2h+1)
                of block b = GRP*g + j."""
                b = GRP * g + j
                hb = hb_ring[g % HB_RING]
                psm = psmp.tile([CH, 2, NSC], f32, tag="psm")
                for c in range(2):
                    ck = 2 * h + c
                    sl = hb[:, j * BLK + ck * CH : j * BLK + (ck + 1) * CH]
                    nc.tensor.matmul(
                        psm[:, c, :], sl, caug[:], start=True, stop=False
                    )
                    nc.tensor.matmul(
                        psm[:, c, :], sl, caug2[:], start=False, stop=True
                    )
                top32 = smallp.tile([CH, 2, 8], f32, tag="top32")
                for c in range(2):
                    nc.vector.max(top32[:, c, :], psm[:, c, 0:N_FCNS])
                tsum = smallp.tile([CH, 2, 1], f32, tag="tsum")
                nc.gpsimd.tensor_add(tsum[:], top32[:, :, 4:5], top32[:, :, 5:6])
                bias4 = smallp.tile([CH, 2, 1], f32, tag="bias4")
                nc.gpsimd.tensor_scalar_mul(bias4[:], tsum[:], -0.5)
                ecp = smallp.tile([CH, 2, 4], f32, tag="ecp")
                nc.vector.tensor_copy(ecp[:], psm[:, :, 250:254])

                S = s_ring[(2 * b + h) % S_RING]
                for c in range(2):
                    nc.scalar.activation(
                        S[:, c, 0:N_FCNS],
                        psm[:, c, 0:N_FCNS],
                        AF.Sign,
                        bias=bias4[:, c, :],
                    )
                return (S, ecp)

            def phase_b(g, j, h, S, ecp):
                """Transposes / G matmuls / combine for chunks (2h, 2h+1)."""
                b = GRP * g + j
                pt4 = pt4p.tile([CH, 2, 6], f32, tag="pt4")
                for c in range(2):
                    pst = pstp.tile([128, 256], fp16, tag="pst")
                    nc.tensor.transpose(pst[:, 0:128], S[:, c, 0:128], identb[:])
                    nc.tensor.transpose(pst[:, 128:256], S[:, c, 128:256], identb[:])
                    st = stp.tile([128, 256], fp16, tag="st")
                    if c == 0:
                        nc.scalar.copy(st[:], pst[:])
                    else:
                        nc.vector.tensor_copy(st[:], pst[:])
                    nc.tensor.matmul(
                        pt4[:, c, :], st[:, 0:128], g1b[:], start=True, stop=False
                    )
                    nc.tensor.matmul(
                        pt4[:, c, :],
                        st[0:123, 128:256],
                        g2b[:],
                        start=False,
                        stop=True,
                    )
                v = smallp.tile([CH, 2, 4], f32, tag="v")
                nc.vector.tensor_mul(v[:], pt4[:, :, 0:4], ecp[:])
                w = smallp.tile([CH, 2, 2], f32, tag="w")
                nc.vector.tensor_add(w[:], v[:, :, 0:2], v[:, :, 2:4])
                m = b % 8
                if m == 0 and h == 0:
                    outstage = outp.tile([CH, 8 * NCH, D_ENC], f32, tag="os")
                    state["outstage"] = outstage
                outstage = state["outstage"]
                nc.vector.scalar_tensor_tensor(
                    outstage[:, 4 * m + 2 * h : 4 * m + 2 * h + 2, :],
                    pt4[:, :, 4:6],
                    2.0 ** -10,
                    w[:],
                    mybir.AluOpType.mult,
                    mybir.AluOpType.add,
                )
                if m == 7 and h == 1:
                    g8 = b // 8
                    nc.sync.dma_start(
                        out_d[:, 32 * g8 : 32 * (g8 + 1), :], outstage[:]
                    )

            import contextlib
            from collections import deque

            QUAD = 4  # groups per pipeline wave
            nquad = ngrp // QUAD

            loop_cm = tc.For_i(0, reps, 1) if reps > 1 else contextlib.nullcontext()
            with loop_cm:
                pending = deque()
                pending_b = deque()
                for t_ in range(0, nquad + 2, 2):
                    quads = [
                        list(range(tt * QUAD, (tt + 1) * QUAD))
                        for tt in (t_, t_ + 1)
                        if tt < nquad
                    ]
                    allg = [g for q_ in quads for g in q_]
                    if allg:
                        for g in allg:
                            hb = hb_ring[g % HB_RING]
                            nc.sync.dma_start(
                                hb[2 : 2 + 2 * D_IN, :],
                                x_fm[:, g * GRP * BLK : (g + 1) * GRP * BLK],
                            )
                        for i in range(N_LAYERS):
                            fi = 33 + 5 * i
                            ki = 2 * fi  # interleaved K rows
                            for sub in quads:
                                pz4 = pzp.tile([128, GRP, BLK], f32, tag="pz")
                                for q, g in enumerate(sub):
                                    hb = hb_ring[g % HB_RING]
                                    for j in range(GRP):
                                        mv = hb[0:ki, j * BLK : (j + 1) * BLK]
                                        nc.tensor.matmul(
                                            pz4[32 * q : 32 * q + 32, j, :],
                                            wt[i][0:ki, :],
                                            mv,
                                            start=True,
                                            stop=False,
                                            tile_position=(0, 32 * q),
                                        )
                                        nc.tensor.matmul(
                                            pz4[32 * q : 32 * q + 32, j, :],
                                            wl[i][0:ki, :],
                                            mv,
                                            start=False,
                                            stop=True,
                                            tile_position=(0, 32 * q),
                                        )
                                hth = htp.tile([101, GRP * BLK], fp16, tag="ht")
                                htf = htfp.tile([101, GRP * BLK], f32, tag="htf")
                                htl = htlp.tile([101, GRP * BLK], fp16, tag="htl")
                                nc.scalar.activation(
                                    hth[:],
                                    pz4[0:101, :, :].rearrange("p a b -> p (a b)"),
                                    AF.Tanh,
                                )
                                nc.scalar.activation(
                                    htf[:],
                                    pz4[0:101, :, :].rearrange("p a b -> p (a b)"),
                                    AF.Tanh,
                                )
                                nc.vector.tensor_sub(htl[:], htf[:], hth[:])
                                for q, g in enumerate(sub):
                                    hb = hb_ring[g % HB_RING]
                                    dsth = hb[2 * fi : 2 * fi + 10, :].rearrange(
                                        "(k s) f -> k s f", s=2
                                    )
                                    nc.sync.dma_start(
                                        dsth[:, 0, :], hth[32 * q : 32 * q + 5, :]
                                    )
                                    nc.sync.dma_start(
                                        dsth[:, 1, :], htl[32 * q : 32 * q + 5, :]
                                    )
                                # interleave phases inside each half-slot too
                                for _ in range(4):
                                    if pending:
                                        key = pending.popleft()
                                        stt = phase_a(*key)
                                        pending_b.append((key, stt))
                                    if len(pending_b) > 1:
                                        key2, st2 = pending_b.popleft()
                                        phase_b(*key2, *st2)

                    # flush remaining phases of the prior wave
                    while pending:
                        key = pending.popleft()
                        stt = phase_a(*key)
                        pending_b.append((key, stt))
                        if len(pending_b) > 1:
                            key2, st2 = pending_b.popleft()
                            phase_b(*key2, *st2)
                    while pending_b:
                        key2, st2 = pending_b.popleft()
                        phase_b(*key2, *st2)
                    for g in allg:
                        for j in range(GRP):
                            for h in range(2):
                                pending.append((g, j, h))

    _split_multi_waits(nc, mybir)
    return nc


def _split_multi_waits(nc, mybir):
    """walrus codegen allows only one sync-wait per instruction; hoist extra
    waits into standalone EventSemaphore instructions on the same engine."""
    k = 0
    for f in nc.m.functions:
        for blk in f.blocks:
            newl = []
            changed = False
            for ins in blk.instructions:
                si = ins.sync_info
                if si is not None and len(si.on_wait) > 1:
                    waits = list(si.on_wait)
                    for w in waits[:-1]:
                        ev = mybir.InstEventSemaphore(
                            name=f"WSPLIT-{k}", ins=[], outs=[]
                        )
                        k += 1
                        ev.engine = ins.engine
                        ev.sync_info = mybir.SyncInfo(on_wait=[w], on_update=[])
                        newl.append(ev)
                    ins.sync_info = mybir.SyncInfo(
                        on_wait=[waits[-1]], on_update=list(si.on_update)
                    )
                    changed = True
                newl.append(ins)
            if changed:
                blk.instructions = newl


def _host_constants(W_list, Wout, bout, ctrs, wts, offsets):
    """Folded constant tables (float64 accumulation, f32/fp16 output)."""
    ctrs64 = ctrs.astype(np.float64)
    Wout64 = Wout.astype(np.float64)
    bout64 = bout.astype(np.float64)
    wts64 = wts.astype(np.float64)
    off64 = offsets.astype(np.float64)

    # caug in baseline feature order: rows 0..56 = features, 57 = ones row.
    caug_old = np.zeros((D_H + 1, NSC), dtype=np.float64)
    caug_old[0:D_H, 0:N_FCNS] = 2.0 * (Wout64.T @ ctrs64.T)
    rrow = 2.0 * (ctrs64 @ bout64) - np.sum(ctrs64 * ctrs64, axis=1)
    caug_old[D_H, 0:N_FCNS] = rrow
    # enc-dup columns [e0, e0, e1, e1]
    sc = 2.0 ** -10
    caug_old[0:D_H, 250] = sc * Wout64.T[:, 0]
    caug_old[0:D_H, 251] = sc * Wout64.T[:, 0]
    caug_old[0:D_H, 252] = sc * Wout64.T[:, 1]
    caug_old[0:D_H, 253] = sc * Wout64.T[:, 1]
    caug_old[D_H, 250] = sc * bout64[0]
    caug_old[D_H, 251] = sc * bout64[0]
    caug_old[D_H, 252] = sc * bout64[1]
    caug_old[D_H, 253] = sc * bout64[1]

    # Reorder rows to the H layout [ones, x(32), tanh(25)], then split into
    # an fp16 hi/lo pair duplicated per interleaved K row (row 2f and 2f+1
    # both carry the same caug row; they pair with Hh_f and Hl_f).
    caugf = np.zeros((D_H + 1, NSC), dtype=np.float64)
    caugf[0] = caug_old[D_H]
    caugf[1:33] = caug_old[0:32]
    caugf[33:58] = caug_old[32:57]
    c1 = caugf.astype(np.float16)
    c2 = (caugf - c1.astype(np.float64)).astype(np.float16)
    cdup1 = np.zeros((2 * (D_H + 1), NSC), dtype=np.float16)
    cdup2 = np.zeros((2 * (D_H + 1), NSC), dtype=np.float16)
    cdup1[0::2] = c1
    cdup1[1::2] = c1
    cdup2[0::2] = c2
    cdup2[1::2] = c2

    # G[c] = [w00, w01, w10, w11, b'0, b'1]; b'_o = off[c,o] - sum_i w[c,i,o]*ctr[c,i]
    G = np.zeros((N_FCNS, 6), dtype=np.float64)
    G[:, 0] = wts64[:, 0, 0]
    G[:, 1] = wts64[:, 0, 1]
    G[:, 2] = wts64[:, 1, 0]
    G[:, 3] = wts64[:, 1, 1]
    bprime = off64 - np.einsum("cio,ci->co", wts64, ctrs64)
    G[:, 4] = bprime[:, 0]
    G[:, 5] = bprime[:, 1]

    g1 = (0.5 * 1024.0 * G[0:128]).astype(np.float16)
    g2 = np.zeros((123, 6), dtype=np.float16)
    g2[0:122] = (0.5 * 1024.0 * G[128:N_FCNS]).astype(np.float16)
    g2[122] = (0.5 * 1024.0 * G.sum(axis=0)).astype(np.float16)

    consts = {
        "cdup1": np.ascontiguousarray(cdup1),
        "cdup2": np.ascontiguousarray(cdup2),
        "g1b": np.ascontiguousarray(g1),
        "g2b": np.ascontiguousarray(g2),
        "identb": np.eye(128, dtype=np.float16),
        "ones1024": np.concatenate(
            [
                np.ones((1, GRP * BLK), dtype=np.float16),
                np.zeros((1, GRP * BLK), dtype=np.float16),
            ]
        ),
    }
    # Layer weight tables, interleaved-K fp16 hi/lo: feature f contributes
    # rows (2f, 2f+1). wt rows (2f, 2f+1) = Wh_f (pairs Hh_f and Hl_f);
    # wl rows = (Wl_f, 0). Feature order: ones, x(32), tanh(5i).
    for i, W in enumerate(W_list):
        fan_in = D_IN + i * GROWTH
        fi = 33 + 5 * i
        wv = np.zeros((fi, GROWTH), dtype=np.float64)
        wv[1:33] = W[:, 0:D_IN].astype(np.float64).T
        if i > 0:
            wv[33:fi] = W[:, D_IN:fan_in].astype(np.float64).T
        wh = wv.astype(np.float16)
        wlo = (wv - wh.astype(np.float64)).astype(np.float16)
        td = np.zeros((2 * fi, 32), dtype=np.float16)
        tl = np.zeros((2 * fi, 32), dtype=np.float16)
        td[0::2, 0:GROWTH] = wh
        td[1::2, 0:GROWTH] = wh
        tl[0::2, 0:GROWTH] = wlo
        consts[f"wt{i}"] = td
        consts[f"wl{i}"] = tl
    return consts


def _in_maps(inputs):
    x = np.asarray(inputs["x"], dtype=np.float32)
    W_list = [np.asarray(inputs[f"W{i}"], dtype=np.float32) for i in range(N_LAYERS)]
    consts = _host_constants(
        W_list,
        np.asarray(inputs["Wout"], dtype=np.float32),
        np.asarray(inputs["bout"], dtype=np.float32),
        np.asarray(inputs["ctrs"], dtype=np.float32),
        np.asarray(inputs["wts"], dtype=np.float32),
        np.asarray(inputs["offsets"], dtype=np.float32),
    )

    xt = x.T.astype(np.float64)  # [32, N_SMPS]
    xh = xt.astype(np.float16)
    xl = (xt - xh.astype(np.float64)).astype(np.float16)
    x_fm = np.empty((2 * D_IN, N_SMPS), dtype=np.float16)
    x_fm[0::2] = xh
    x_fm[1::2] = xl
    in_maps = []
    for core in range(N_CORES):
        m = dict(consts)
        m["x_fm"] = np.ascontiguousarray(x_fm[:, core * N_PER : (core + 1) * N_PER])
        in_maps.append(m)
    return in_maps


def _unstage(arr):
    """[128, 512, 2] p-major stage layout -> [65536, 2] sample-major."""
    return np.ascontiguousarray(
        arr.reshape(CH, NBLK, NCH, D_ENC).transpose(1, 2, 0, 3).reshape(N_PER, D_ENC)
    )


def _run(inputs, trace=False, nblk=NBLK):
    from concourse.bass_utils import run_bass_kernel_spmd

    key = ("nc", nblk)
    if key not in _cache:
        _cache[key] = _build_program(nblk)
    nc = _cache[key]

    in_maps = _in_maps(inputs)

    res = run_bass_kernel_spmd(nc, in_maps, list(range(N_CORES)), trace=trace)
    outs = [_unstage(res.results[c]["out"]) for c in range(N_CORES)]
    full = np.concatenate(outs, axis=0).astype(np.float32)
    return full, res


def kernel(**inputs):
    full, _ = _run(inputs, trace=False)
    return full


if __name__ == "__main__":
    rng = np.random.default_rng(0)
    demo = {"x": rng.standard_normal((N_SMPS, D_IN), dtype=np.float32)}
    for i in range(N_LAYERS):
        fan_in = D_IN + i * GROWTH
        demo[f"W{i}"] = rng.standard_normal((GROWTH, fan_in), dtype=np.float32) * 0.1
    demo["Wout"] = rng.standard_normal((D_ENC, D_H), dtype=np.float32) * 0.1
    demo["bout"] = rng.standard_normal(D_ENC, dtype=np.float32) * 0.1
    demo["ctrs"] = rng.standard_normal((N_FCNS, D_ENC), dtype=np.float32)
    demo["wts"] = 1e-5 * rng.standard_normal((N_FCNS, D_ENC, D_ENC), dtype=np.float32)
    demo["offsets"] = 1e-5 * rng.standard_normal((N_FCNS, D_ENC), dtype=np.float32)
    out = kernel(**demo)
    print(out.shape, out.dtype)
